# revision 42
# baseline (speedup 1.0000x reference)
"""Trainium2 Bass kernel for a codec-transformer block (sliding-window GQA + SwiGLU).

Sharding: data-parallel over 8 token chunks (2 batches x 4 chunks of 512
tokens). The 512-token sliding window makes attention local: each core
receives its 512 "own" tokens plus the preceding 512 tokens as a KV halo,
so no collectives are needed.

Host-side prep (layout only, no model FLOPs):
  - attn_norm_w folded into wq/wk/wv columns, ffn_norm_w into w1/w3 columns
    (rmsnorm weight commutes into the following matmul exactly)
  - attn_scale folded into wo rows, ffn_scale into w2 rows (layer-scale
    commutes out of the preceding matmul exactly)
  - weights transposed to [in, out] and cast to bf16 (all matmuls run in
    bf16 with fp32 PSUM accumulation; the residual path stays fp32, and both
    branch outputs are damped by the 0.01 layer-scales so bf16 branch error
    is ~1e-4 relative on the final output)
  - per-core sliding-window masks (multiplicative, on exp(scores))

On-chip dataflow is token-major; PE transpose (via identity matmul) produces
the feature-major operands needed for stationary/moving tensors. Softmax is
computed without max-subtraction (rmsnorm'd q,k bound |score| <= 8) and
unnormalized; a ones-column appended to V yields sum(exp) in the same PSUM
tile, and the reciprocal is applied per-partition at eviction. Engine streams
execute in emission order, so norm chains are software-pipelined two tiles
ahead of their consumers to keep the PE dense (HAM stays warm).
"""

import os
import sys

sys.path.insert(0, "/opt/trn_rl_repo")
os.environ.setdefault("MYCRO_LOCAL_CACHE", "1")

from contextlib import ExitStack

import numpy as np
import ml_dtypes

import concourse.bass as bass
import concourse.bacc as bacc
import concourse.tile as tile
from concourse import mybir
from concourse.masks import make_identity
from concourse.bass_utils import run_bass_kernel_spmd

BF16 = mybir.dt.bfloat16
F32 = mybir.dt.float32
F8 = mybir.dt.float8e4
AF = mybir.ActivationFunctionType
ALU = mybir.AluOpType
DR = mybir.MatmulPerfMode.DoubleRow
NPBF16 = ml_dtypes.bfloat16
NPF8 = ml_dtypes.float8_e4m3

# fp8 scaling: weights are upscaled into e4m3's normal range (min normal
# 2^-6) and the product rescaled at eviction.  qk-norm makes q/k scale
# invariant; V/WO/FFN evictions fold the downscale into an existing fused op.
WUP = 32.0           # wq/wk/wv/w1/w3 upscale (values ~N(0,1) in fp8)
WOUP = 4096.0        # wo*attn_scale / w2*ffn_scale upscale
RES_SC = 1.0 / WOUP  # residual-add eviction scale
FT_SC = 1.0 / (WUP * WUP)  # silu(g)*u rescale (psg,psu both carry 32x)

P = 128
B, T, D = 2, 2048, 1024
HID = 4096
H, KVH, HD = 16, 4, 64
KD = D // P            # 8 contraction tiles over model dim
KH = HID // P          # 32 contraction tiles over hidden dim
OWN = 512              # tokens owned per core
CTX = 1024             # own + 512-token halo
NQT = OWN // P         # 4
NKT = CTX // P         # 8
NCORES = 8
KC = KVH * HD          # 256
EPS = 1e-5
QKEPS = 1e-6
SM_SCALE = 1.0 / 8.0   # 1/sqrt(HD)


def _qclip(ki):
    """Valid own-query range for ctx key tile ki under the sliding window."""
    return max(0, P * (ki - 4)), min(OWN, P * (ki + 1))


# Packed per-head scores layout: (ki, qlo, qhi, col_offset) blocks.  Bank 0
# holds the four diagonal (ki==qt) blocks, bank 1 the four window-edge
# (ki==qt+4) blocks, banks 2-4 the interior blocks.  Every block sits inside
# one 512-float PSUM bank.
SPACK = [
    (0, 0, 128, 0), (1, 128, 256, 128), (2, 256, 384, 256), (3, 384, 512, 384),
    (4, 0, 128, 512), (5, 128, 256, 640), (6, 256, 384, 768),
    (7, 384, 512, 896),
    (3, 0, 384, 1024), (1, 0, 128, 1408),
    (4, 128, 512, 1536), (6, 384, 512, 1920),
    (2, 0, 256, 2048), (5, 256, 512, 2304),
]
SCOL = 2560
# (ki, qt) -> packed column of that 128-wide block
SLOOKUP = {}
for _ki, _qlo, _qhi, _off in SPACK:
    for _qt in range(_qlo // P, _qhi // P):
        SLOOKUP[(_ki, _qt)] = _off + _qt * P - _qlo
assert len(SLOOKUP) == 20


def _build_tile_kernel(ctx: ExitStack, tc: tile.TileContext, io: dict):
    nc = tc.nc
    xtok, y = io["xtok"], io["y"]

    const = ctx.enter_context(tc.tile_pool(name="const", bufs=1))
    identity = const.tile([P, P], BF16)
    make_identity(nc, identity)
    qw_sb = const.tile([P, HD], F32)
    nc.sync.dma_start(qw_sb, io["qw"])
    kw_sb = const.tile([P, HD], F32)
    nc.sync.dma_start(kw_sb, io["kw"])
    eps_sb = const.tile([P, 1], F32)
    nc.vector.memset(eps_sb, EPS)
    qkeps_sb = const.tile([P, 1], F32)
    nc.vector.memset(qkeps_sb, QKEPS)
    ones_row = const.tile([1, HD], BF16)
    nc.vector.memset(ones_row, 1.0)
    zero_row = const.tile([1, OWN], BF16)
    nc.vector.memset(zero_row, 0.0)

    sstat = ctx.enter_context(tc.tile_pool(name="sstat", bufs=8))

    # persistent activations; pools are ordered so releases stay LIFO:
    # open order pers < hnT < attn_pers < xall < maskp < attnT < stage pools
    pers = ctx.enter_context(tc.tile_pool(name="pers", bufs=1))
    h_sb = pers.tile([P, NQT, D], F32)       # residual h = x + r, fp32
    hnT_pool = ctx.enter_context(tc.tile_pool(name="hnT_pool", bufs=1))
    hnT = hnT_pool.tile([P, KD, OWN], F8)
    ap_stack = ExitStack()
    attn_pers = ap_stack.enter_context(tc.tile_pool(name="attn_pers", bufs=1))
    # qhat^T: q heads are laid out (via the host-side wq column permutation)
    # so head h lives in feature tile tau=(h%4)+4*(h//8) at partition base
    # pi=((h//4)%2)*64 -- exactly where its kv head lands in kT2's natural
    # pair-transpose layout, so scores operands always share a base partition.
    qkT = attn_pers.tile([P, KD, OWN], BF16)
    # khat^T: pair-transposed [2 kv heads x 64, ctx] per feature tile
    kT2 = attn_pers.tile([P, 2, CTX], BF16)
    # v tokens + per-key VALIDITY column: halo keys of the first chunk carry
    # validity 0, so they drop out of both the PV numerator (v rows are zero
    # there anyway) and the softmax denominator -- no per-chunk masks needed.
    v65 = attn_pers.tile([P, NKT, KVH, HD + 1], BF16)
    nc.sync.dma_start(v65[:, :, :, HD:HD + 1], io["valid"])

    # Prefetch the whole x slice up front: 8 parallel DMAs overlap with the
    # first norm chains; the own-token tiles also serve the stage-D residual.
    xall_stack = ExitStack()
    xall_pool = xall_stack.enter_context(tc.tile_pool(name="xall", bufs=1))
    xall = xall_pool.tile([P, NKT, D], F32)
    # first two tiles split into 4 chunk-DMAs each so the stage-A pipeline
    # starts ~3us in instead of waiting ~14us for a whole-tile transfer
    for i in range(2):
        for c in range(4):
            nc.sync.dma_start(xall[:, i, c * 256:(c + 1) * 256],
                              xtok[i * P:(i + 1) * P, c * 256:(c + 1) * 256])
    for i in range(2, NKT):
        nc.sync.dma_start(xall[:, i, :], xtok[i * P:(i + 1) * P, :])

    # ---- Stages A+B: rmsnorm + xnT transpose + QKV, per ctx tile ----
    stage_a = ExitStack()
    with stage_a:
        wqkv_pool = stage_a.enter_context(tc.tile_pool(name="wqkv", bufs=1))
        wkv_sb = wqkv_pool.tile([P, KD, 2 * KC], F8)
        nc.sync.dma_start(wkv_sb, io["wkvT"].rearrange("(kd p) n -> p kd n", p=P))
        wq_sb = wqkv_pool.tile([P, KD, D], F8)
        nc.sync.dma_start(wq_sb, io["wqT"].rearrange("(kd p) n -> p kd n", p=P))

        pa = stage_a.enter_context(tc.tile_pool(name="pa", bufs=3))
        tp_ps = stage_a.enter_context(
            tc.tile_pool(name="tp_ps", bufs=4, space="PSUM"))
        xnT_pool = stage_a.enter_context(tc.tile_pool(name="xnT_pool", bufs=1))
        xnT = xnT_pool.tile([P, KD, CTX], F8)
        pb_ps = stage_a.enter_context(
            tc.tile_pool(name="pb_ps", bufs=3, space="PSUM"))
        pb = stage_a.enter_context(tc.tile_pool(name="pb", bufs=2))

        def norm_tile(i):
            """rmsnorm of x tile i -> bf16 xn tile (ACT/DVE only)."""
            xt = xall[:, i, :]
            sq = pa.tile([P, D], F32, tag="sq")
            ssq = sstat.tile([P, 1], F32, tag="ssq")
            nc.scalar.activation(sq, xt, AF.Square, accum_out=ssq)
            std = sstat.tile([P, 1], F32, tag="std")
            nc.scalar.activation(std, ssq, AF.Sqrt, bias=eps_sb, scale=1.0 / D)
            rstd = sstat.tile([P, 1], F32, tag="rstd")
            nc.vector.reciprocal(rstd, std)
            xn = pa.tile([P, D], BF16, tag="xn")
            nc.vector.tensor_scalar_mul(xn, xt, rstd)
            return xn

        def emit_k_tp(kt, khat):
            # pair transpose: kv heads 2kf (base 0) and 2kf+1 (base 64)
            for kf in range(2):
                pt = tp_ps.tile([P, P], BF16, tag="tp")
                nc.tensor.transpose(pt, khat[:, kf * P:(kf + 1) * P], identity)
                nc.vector.tensor_copy(kT2[:, kf, kt * P:(kt + 1) * P], pt)

        def emit_q_tp(qt, qhats):
            for half in range(2):
                for j in range(4):
                    pt = tp_ps.tile([P, P], BF16, tag="tp")
                    nc.tensor.transpose(
                        pt, qhats[half][:, j * P:(j + 1) * P], identity)
                    nc.vector.tensor_copy(
                        qkT[:, half * 4 + j, qt * P:(qt + 1) * P], pt)

        # Norm chains run two tiles ahead of the PE; k/q-hat transposes run
        # one tile BEHIND their matmuls so the qk-norm ACT/DVE chains finish
        # in the shadow of the next tile's matmuls.
        xns = {0: norm_tile(0), 1: norm_tile(1)}
        pending_k = None
        pending_q = None
        for i in range(NKT):
            xn = xns.pop(i)
            for kd in range(KD):
                pt = tp_ps.tile([P, P], BF16, tag="tp")
                nc.tensor.transpose(pt, xn[:, kd * P:(kd + 1) * P], identity)
                nc.vector.tensor_copy(xnT[:, kd, i * P:(i + 1) * P], pt)
            if i + 2 < NKT:
                xns[i + 2] = norm_tile(i + 2)

            # K / V projection for ctx tile i (fp8 DoubleRow: 2 kd per MM)
            kt = i
            ps = pb_ps.tile([P, 512], F32, tag="ps")
            for kp in range(KD // 2):
                nc.tensor.matmul(
                    ps, lhsT=xnT[:, 2 * kp:2 * kp + 2, kt * P:(kt + 1) * P],
                    rhs=wkv_sb[:, 2 * kp:2 * kp + 2, :],
                    start=(kp == 0), stop=(kp == KD // 2 - 1), perf_mode=DR)
            if pending_k is not None:
                emit_k_tp(*pending_k)
            sqk = pb.tile([P, KC], F32, tag="sqk")
            nc.scalar.activation(sqk, ps[:, 0:KC], AF.Square)
            msk = pb.tile([P, KVH], F32, tag="msk")
            nc.vector.reduce_sum(
                msk, sqk.rearrange("p (h e) -> p h e", e=HD),
                axis=mybir.AxisListType.X)
            sck = sstat.tile([P, KVH], F32, tag="sck")
            nc.scalar.activation(sck, msk, AF.Sqrt, bias=qkeps_sb, scale=1.0 / HD)
            rck = sstat.tile([P, KVH], F32, tag="rck")
            nc.vector.reciprocal(rck, sck)
            tmk = pb.tile([P, KC], F32, tag="tmk")
            nc.vector.tensor_mul(
                tmk.rearrange("p (h e) -> p h e", e=HD),
                ps[:, 0:KC].rearrange("p (h e) -> p h e", e=HD),
                kw_sb[:, None, :].broadcast_to([P, KVH, HD]))
            khat = pb.tile([P, KC], BF16, tag="khat")
            nc.vector.tensor_mul(
                khat.rearrange("p (h e) -> p h e", e=HD),
                tmk.rearrange("p (h e) -> p h e", e=HD),
                rck[:, :, None].broadcast_to([P, KVH, HD]))
            pending_k = (kt, khat)
            nc.scalar.activation(
                v65[:, kt, :, 0:HD],
                ps[:, KC:2 * KC].rearrange("p (h e) -> p h e", e=HD),
                AF.Copy, scale=1.0 / WUP)

            # Q projection + qk-norm for own tile qt = i - 4
            if i < NQT:
                continue
            qt = i - NQT
            col = OWN + qt * P
            msq = pb.tile([P, H], F32, tag="msq")
            pss = []
            for half in range(2):
                ps = pb_ps.tile([P, 512], F32, tag="ps")
                pss.append(ps)
                for kp in range(KD // 2):
                    nc.tensor.matmul(
                        ps, lhsT=xnT[:, 2 * kp:2 * kp + 2, col:col + P],
                        rhs=wq_sb[:, 2 * kp:2 * kp + 2,
                                  half * 512:(half + 1) * 512],
                        start=(kp == 0), stop=(kp == KD // 2 - 1),
                        perf_mode=DR)
                sqq = pb.tile([P, 512], F32, tag="sqq")
                nc.scalar.activation(sqq, ps, AF.Square)
                nc.vector.reduce_sum(
                    msq[:, half * 8:(half + 1) * 8],
                    sqq.rearrange("p (h e) -> p h e", e=HD),
                    axis=mybir.AxisListType.X)
            if pending_q is not None:
                emit_q_tp(*pending_q)
            sc = sstat.tile([P, H], F32, tag="sc")
            nc.scalar.activation(sc, msq, AF.Sqrt, bias=qkeps_sb, scale=1.0 / HD)
            rc = sstat.tile([P, H], F32, tag="rc")
            nc.vector.reciprocal(rc, sc)
            qhats = []
            for half in range(2):
                ps = pss[half]
                tmq = pb.tile([P, 512], F32, tag="tmq")
                nc.vector.tensor_mul(
                    tmq.rearrange("p (h e) -> p h e", e=HD),
                    ps.rearrange("p (h e) -> p h e", e=HD),
                    qw_sb[:, None, :].broadcast_to([P, 8, HD]))
                qhat = pb.tile([P, 512], BF16, tag="qhat")
                nc.vector.tensor_mul(
                    qhat.rearrange("p (h e) -> p h e", e=HD),
                    tmq.rearrange("p (h e) -> p h e", e=HD),
                    rc[:, half * 8:(half + 1) * 8, None].broadcast_to([P, 8, HD]))
                qhats.append(qhat)
            pending_q = (qt, qhats)
        emit_k_tp(*pending_k)
        emit_q_tp(*pending_q)

    # ---- Stage C: attention (packed scores PSUM + PV-swap) ----
    # Scores for one head are packed into a single 5-bank PSUM tile so exp
    # runs as 2 big ACT ops (the 352-cycle ACT overhead dominated the old
    # per-ki exp).  Layout regions: bank0 = the 4 diagonal (ki==qt) blocks,
    # bank1 = the 4 window-edge (ki==qt+4) blocks, banks 2-4 = interior
    # blocks (always fully valid for interior chunks).  One full-width mask
    # multiply per head handles both boundary triangles and the chunk-0 halo.
    # PV is swapped vs the baseline: v65 is the stationary operand and eS the
    # moving one, so the output lands directly feature-major as attnT (no PE
    # transposes) and the per-MM cost drops from ~110ns (LDW-bound, N=65) to
    # ~60ns (N=128).  The ones column of v65 lands in PSUM partition 64; its
    # reciprocal (DVE, also the PSUM->SBUF move) is partition-broadcast by
    # GpSimd and applied in a single [64,512] eviction multiply per head.
    mask_stack = ExitStack()
    maskp = mask_stack.enter_context(tc.tile_pool(name="maskp", bufs=1))
    masks_sb = maskp.tile([P, 2 * OWN], BF16)
    nc.sync.dma_start(masks_sb, io["masks"])
    attnT_stack = ExitStack()
    attnT_pool = attnT_stack.enter_context(tc.tile_pool(name="attnT_pool",
                                                        bufs=1))
    attnT = attnT_pool.tile([P, KD, OWN], F8)

    stage_c = ExitStack()
    with stage_c:
        pc_ex = stage_c.enter_context(tc.tile_pool(name="pc_ex", bufs=2))
        pc_es = stage_c.enter_context(tc.tile_pool(name="pc_es", bufs=2))
        pc_rec = stage_c.enter_context(tc.tile_pool(name="pc_rec", bufs=2))
        ps_s = stage_c.enter_context(
            tc.tile_pool(name="ps_s", bufs=1, space="PSUM"))
        ps_o = stage_c.enter_context(
            tc.tile_pool(name="ps_o", bufs=2, space="PSUM"))
        ps_b = stage_c.enter_context(
            tc.tile_pool(name="ps_b", bufs=1, space="PSUM"))
        # two separate tiles (diag+edge / interior) so head h's boundary
        # scores can start as soon as exp1(h-1) has read its region, without
        # waiting for exp2(h-1)
        SA = ps_s.tile([P, 2 * OWN], F32)   # banks for diag+edge regions
        SB = ps_s.tile([P, SCOL - 2 * OWN], F32)  # interior regions

        def emit_warm():
            # Full-width dummy matmul (K=128, N=512).  The attention MM mix
            # (64-deep scores, N=128 PV) never trips HAM's activity monitor,
            # so the whole phase runs at 1.2 GHz; a few dense matmuls per
            # head keep the clock gate at 8/8.
            wps = ps_b.tile([P, OWN], F32, tag="sb")
            nc.tensor.matmul(wps, lhsT=identity, rhs=masks_sb[:, 0:OWN],
                             start=True, stop=True)

        def emit_scores(h):
            tau = (h % 4) + 4 * (h // 8)
            kf = (h // 4) // 2
            pi = ((h // 4) % 2) * HD
            for n, (ki, qlo, qhi, off) in enumerate(SPACK):
                w = qhi - qlo
                if n == 8:
                    emit_warm()
                dst = SA[:, off:off + w] if off < 2 * OWN else \
                    SB[:, off - 2 * OWN:off - 2 * OWN + w]
                nc.tensor.matmul(
                    dst,
                    lhsT=kT2[pi:pi + HD, kf, ki * P:(ki + 1) * P],
                    rhs=qkT[pi:pi + HD, tau, qlo:qhi],
                    start=True, stop=True)
            emit_warm()

        def emit_exp_mask(h):
            # diag+edge regions [0:1024) get the (constant) triangle mask;
            # interior regions exp straight into eS
            ex = pc_ex.tile([P, 2 * OWN], BF16, tag="ex")
            eS = pc_es.tile([P, SCOL], BF16, tag="eS")
            nc.scalar.activation(ex, SA, AF.Exp, scale=SM_SCALE)
            nc.scalar.activation(eS[:, 2 * OWN:SCOL], SB,
                                 AF.Exp, scale=SM_SCALE)
            nc.vector.tensor_mul(eS[:, 0:2 * OWN], ex, masks_sb)
            return eS

        def emit_pv(h, eS):
            kvh = h // 4
            tau = (h % 4) + 4 * (h // 8)
            slot = 2 * tau + ((h // 4) % 2)
            pi = (slot % 2) * HD
            kd = slot // 2
            # ki-major with merged interior blocks.  start=True resets the
            # whole 2KB PSUM bank (not just the written window), so a K=1
            # zeroing matmul opens the bank once and every PV matmul
            # accumulates; only the last carries stop.
            po = ps_o.tile([P, OWN], F32, tag="po")
            nc.tensor.matmul(po[0:HD + 1, :], lhsT=zero_row[0:1, 0:HD + 1],
                             rhs=zero_row[0:1, :], start=True, stop=False,
                             skip_group_check=True)
            for ki in range(NKT):
                if ki == 4:
                    emit_warm()
                if ki <= 3:  # diagonal block (window qt=ki)
                    nc.tensor.matmul(
                        po[0:HD + 1, ki * P:(ki + 1) * P],
                        lhsT=v65[:, ki, kvh, :],
                        rhs=eS[:, SLOOKUP[(ki, ki)]:SLOOKUP[(ki, ki)] + P],
                        start=False, stop=False, skip_group_check=True)
                if ki <= 3:        # interior = qt 0..ki-1 (diag qt=ki is last)
                    ilo, ihi = 0, ki * P
                else:              # interior = qt ki-3..3 (edge qt=ki-4 first)
                    ilo, ihi = (ki - 3) * P, OWN
                if ihi > ilo:
                    col = SLOOKUP[(ki, ilo // P)]
                    nc.tensor.matmul(
                        po[0:HD + 1, ilo:ihi],
                        lhsT=v65[:, ki, kvh, :],
                        rhs=eS[:, col:col + ihi - ilo],
                        start=False, stop=False, skip_group_check=True)
                if ki >= 4:  # edge block (window qt=ki-4)
                    qt = ki - 4
                    nc.tensor.matmul(
                        po[0:HD + 1, qt * P:(qt + 1) * P],
                        lhsT=v65[:, ki, kvh, :],
                        rhs=eS[:, SLOOKUP[(ki, qt)]:SLOOKUP[(ki, qt)] + P],
                        start=False, stop=(ki == NKT - 1),
                        skip_group_check=True)
            # sums row -> SBUF (bf16), PE K=1 broadcast to 64 partitions,
            # reciprocal back to SBUF, one multiply into attnT
            srow = pc_rec.tile([1, OWN], BF16, tag="srow")
            nc.vector.tensor_copy(srow[0:1, :], po[HD:HD + 1, :])
            sb = ps_b.tile([P, OWN], F32, tag="sb")
            nc.tensor.matmul(sb[0:HD, :], lhsT=ones_row[0:1, :],
                             rhs=srow[0:1, :], start=True, stop=True)
            rec_b = pc_rec.tile([HD, OWN], F32, tag="rec_b")
            nc.vector.reciprocal_approx_fast(out=rec_b, in_=sb[0:HD, :])
            nc.vector.tensor_mul(attnT[pi:pi + HD, kd, :], po[0:HD, :], rec_b)

        pending = None
        for h in range(H):
            emit_scores(h)
            eS = emit_exp_mask(h)
            if pending is not None:
                emit_pv(*pending)
            pending = (h, eS)
        emit_pv(*pending)

    # ---- Stages D+E: output projection + residual + ffn norm, per qt ----
    stage_de = ExitStack()
    with stage_de:
        wo_pool = stage_de.enter_context(tc.tile_pool(name="wo_pool", bufs=1))
        wo_sb = wo_pool.tile([P, KD, D], F8)
        nc.sync.dma_start(wo_sb, io["woT"].rearrange("(kd p) n -> p kd n", p=P))
        ps_r = stage_de.enter_context(
            tc.tile_pool(name="ps_r", bufs=2, space="PSUM"))
        tp_ps3 = stage_de.enter_context(
            tc.tile_pool(name="tp_ps3", bufs=2, space="PSUM"))
        pe = stage_de.enter_context(tc.tile_pool(name="pe", bufs=2))

        for qt in range(NQT):
            xr = xall[:, NQT + qt, :]
            for half in range(2):
                ps = ps_r.tile([P, 512], F32, tag="psr")
                for kp in range(KD // 2):
                    nc.tensor.matmul(
                        ps, lhsT=attnT[:, 2 * kp:2 * kp + 2, qt * P:(qt + 1) * P],
                        rhs=wo_sb[:, 2 * kp:2 * kp + 2,
                                  half * 512:(half + 1) * 512],
                        start=(kp == 0), stop=(kp == KD // 2 - 1),
                        perf_mode=DR)
                # h = ps * (attn_scale/WOUP already in wo) ... = ps/WOUP + x
                nc.vector.scalar_tensor_tensor(
                    h_sb[:, qt, half * 512:(half + 1) * 512], ps, RES_SC,
                    xr[:, half * 512:(half + 1) * 512], ALU.mult, ALU.add)
            # ffn rmsnorm for this qt (overlaps next qt's wo matmuls)
            sqh = pe.tile([P, D], F32, tag="sqh")
            ssqh = sstat.tile([P, 1], F32, tag="ssq")
            nc.scalar.activation(sqh, h_sb[:, qt, :], AF.Square, accum_out=ssqh)
            stdh = sstat.tile([P, 1], F32, tag="std")
            nc.scalar.activation(stdh, ssqh, AF.Sqrt, bias=eps_sb, scale=1.0 / D)
            rstdh = sstat.tile([P, 1], F32, tag="rstd")
            nc.vector.reciprocal(rstdh, stdh)
            hn = pe.tile([P, D], BF16, tag="hn")
            nc.vector.tensor_scalar_mul(hn, h_sb[:, qt, :], rstdh)
            for kd in range(KD):
                pt = tp_ps3.tile([P, P], BF16, tag="tp3")
                nc.tensor.transpose(pt, hn[:, kd * P:(kd + 1) * P], identity)
                nc.vector.tensor_copy(hnT[:, kd, qt * P:(qt + 1) * P], pt)

    attnT_stack.close()
    mask_stack.close()
    xall_stack.close()
    ap_stack.close()

    # ---- Stage F: SwiGLU FFN ----
    stage_f = ExitStack()
    with stage_f:
        fT_pool = stage_f.enter_context(tc.tile_pool(name="fT_pool", bufs=1))
        fT = fT_pool.tile([P, KH, OWN], F8)     # (silu(g) * u)^T feature-major
        w13 = stage_f.enter_context(tc.tile_pool(name="w13", bufs=4))
        ps_f = stage_f.enter_context(
            tc.tile_pool(name="ps_f", bufs=2, space="PSUM"))
        pf = stage_f.enter_context(tc.tile_pool(name="pf", bufs=2))

        for mi in range(KH):
            w1t = w13.tile([P, KD, P], F8, tag="w1t")
            nc.sync.dma_start(
                w1t, io["w1T"][:, mi * P:(mi + 1) * P]
                .rearrange("(kd p) m -> p kd m", p=P))
            w3t = w13.tile([P, KD, P], F8, tag="w3t")
            nc.sync.dma_start(
                w3t, io["w3T"][:, mi * P:(mi + 1) * P]
                .rearrange("(kd p) m -> p kd m", p=P))
            psg = ps_f.tile([P, 512], F32, tag="pg")
            for kp in range(KD // 2):
                nc.tensor.matmul(psg, lhsT=w1t[:, 2 * kp:2 * kp + 2, :],
                                 rhs=hnT[:, 2 * kp:2 * kp + 2, :],
                                 start=(kp == 0), stop=(kp == KD // 2 - 1),
                                 perf_mode=DR)
            psu = ps_f.tile([P, 512], F32, tag="pu")
            for kp in range(KD // 2):
                nc.tensor.matmul(psu, lhsT=w3t[:, 2 * kp:2 * kp + 2, :],
                                 rhs=hnT[:, 2 * kp:2 * kp + 2, :],
                                 start=(kp == 0), stop=(kp == KD // 2 - 1),
                                 perf_mode=DR)
            # silu(g)*u via sigmoid; psg/psu carry the WUP upscale: the STT
            # folds the full 1/WUP^2 rescale into gm (tensor_tensor_reduce
            # with fp8 output crashes the device, so rescale upstream)
            sg = pf.tile([P, 512], F32, tag="sg")
            nc.scalar.activation(sg, psg, AF.Sigmoid, scale=1.0 / WUP)
            gm = pf.tile([P, 512], BF16, tag="gm")
            nc.vector.scalar_tensor_tensor(gm, psg, FT_SC, sg, ALU.mult,
                                           ALU.mult)
            nc.vector.tensor_mul(fT[:, mi, :], gm, psu)

        w2_pool = stage_f.enter_context(tc.tile_pool(name="w2_pool", bufs=1))
        w2_sb = w2_pool.tile([P, KH, D], F8)
        # 8 chunked DMAs spread across queues (a single 8MB transfer
        # serializes on one queue and stalls the w2 matmuls ~15us)
        for c in range(8):
            nc.sync.dma_start(
                w2_sb[:, c * 4:(c + 1) * 4, :],
                io["w2T"][c * 4 * P:(c + 1) * 4 * P, :]
                .rearrange("(kh p) n -> p kh n", p=P))
        ps_y = stage_f.enter_context(
            tc.tile_pool(name="ps_y", bufs=2, space="PSUM"))
        py = stage_f.enter_context(tc.tile_pool(name="py", bufs=2))

        for qt in range(NQT):
            yt = py.tile([P, D], F32, tag="yt")
            for half in range(2):
                ps = ps_y.tile([P, 512], F32, tag="psy")
                for kp in range(KH // 2):
                    nc.tensor.matmul(
                        ps, lhsT=fT[:, 2 * kp:2 * kp + 2, qt * P:(qt + 1) * P],
                        rhs=w2_sb[:, 2 * kp:2 * kp + 2,
                                  half * 512:(half + 1) * 512],
                        start=(kp == 0), stop=(kp == KH // 2 - 1),
                        perf_mode=DR)
                nc.vector.scalar_tensor_tensor(
                    yt[:, half * 512:(half + 1) * 512], ps, RES_SC,
                    h_sb[:, qt, half * 512:(half + 1) * 512],
                    ALU.mult, ALU.add)
            nc.sync.dma_start(y[qt * P:(qt + 1) * P, :], yt)


def build_nc():
    nc = bacc.Bacc("TRN2", target_bir_lowering=False, debug=False,
                   num_devices=NCORES)
    io = {
        "xtok": nc.dram_tensor("xtok", [CTX, D], F32, kind="ExternalInput").ap(),
        "wqT": nc.dram_tensor("wqT", [D, D], F8, kind="ExternalInput").ap(),
        "wkvT": nc.dram_tensor("wkvT", [D, 2 * KVH * HD], F8,
                               kind="ExternalInput").ap(),
        "woT": nc.dram_tensor("woT", [D, D], F8, kind="ExternalInput").ap(),
        "w1T": nc.dram_tensor("w1T", [D, HID], F8, kind="ExternalInput").ap(),
        "w3T": nc.dram_tensor("w3T", [D, HID], F8, kind="ExternalInput").ap(),
        "w2T": nc.dram_tensor("w2T", [HID, D], F8, kind="ExternalInput").ap(),
        "qw": nc.dram_tensor("qw", [P, HD], F32, kind="ExternalInput").ap(),
        "kw": nc.dram_tensor("kw", [P, HD], F32, kind="ExternalInput").ap(),
        "masks": nc.dram_tensor("masks", [P, 2 * OWN], BF16,
                                kind="ExternalInput").ap(),
        "valid": nc.dram_tensor("valid", [P, NKT, KVH, 1], BF16,
                                kind="ExternalInput").ap(),
        "y": nc.dram_tensor("y", [OWN, D], F32, kind="ExternalOutput").ap(),
    }
    with tile.TileContext(nc) as tc:
        with ExitStack() as ctx:
            _build_tile_kernel(ctx, tc, io)
    nc.compile()
    return nc


_CACHE = {}


def get_nc():
    if "nc" not in _CACHE:
        _CACHE["nc"] = build_nc()
    return _CACHE["nc"]


def prep_in_maps(inputs):
    """Fold scales into weights, transpose/cast, and slice per-core inputs."""
    f32 = np.float32
    x = np.asarray(inputs["x"], f32)
    wq = np.asarray(inputs["wq"], f32)
    wk = np.asarray(inputs["wk"], f32)
    wv = np.asarray(inputs["wv"], f32)
    wo = np.asarray(inputs["wo"], f32)
    w1 = np.asarray(inputs["w1"], f32)
    w2 = np.asarray(inputs["w2"], f32)
    w3 = np.asarray(inputs["w3"], f32)
    qw = np.asarray(inputs["q_norm_w"], f32)
    kw = np.asarray(inputs["k_norm_w"], f32)
    anw = np.asarray(inputs["attn_norm_w"], f32)
    fnw = np.asarray(inputs["ffn_norm_w"], f32)
    asc = np.asarray(inputs["attn_scale"], f32)
    fsc = np.asarray(inputs["ffn_scale"], f32)

    # q-head permutation: slot j of the on-chip q/attn feature layout holds
    # head HEAD_PERM[j], so each q head's partition half matches its kv
    # head's natural pair-transpose position (see kernel layout comment)
    HEAD_PERM = [0, 4, 1, 5, 2, 6, 3, 7, 8, 12, 9, 13, 10, 14, 11, 15]
    wq_p = (wq * anw[None, :] * WUP).reshape(H, HD, D)[HEAD_PERM] \
        .reshape(H * HD, D)
    wqT = np.ascontiguousarray(wq_p.T).astype(NPF8)
    wkvT = np.ascontiguousarray(
        np.concatenate([wk * anw[None, :], wv * anw[None, :]], axis=0).T * WUP
    ).astype(NPF8)
    wo_p = (wo * asc[:, None] * WOUP).T.reshape(H, HD, D)[HEAD_PERM] \
        .reshape(H * HD, D)
    woT = np.ascontiguousarray(wo_p).astype(NPF8)
    w1T = np.ascontiguousarray((w1 * fnw[None, :] * WUP).T).astype(NPF8)
    w3T = np.ascontiguousarray((w3 * fnw[None, :] * WUP).T).astype(NPF8)
    w2T = np.ascontiguousarray((w2 * fsc[:, None] * WOUP).T).astype(NPF8)
    qwb = np.ascontiguousarray(np.broadcast_to(qw[None, :], (P, HD))).astype(f32)
    kwb = np.ascontiguousarray(np.broadcast_to(kw[None, :], (P, HD))).astype(f32)

    # Boundary mask for the packed diag+edge regions (identical across cores
    # and chunks): diag blocks (ki==qt) are the strict-causal triangle p > j,
    # edge blocks (ki==qt+4) are the window triangle p <= j.  Chunk-0 halo
    # invalidity is handled by the v65 validity column, not the mask.
    p_idx = np.arange(P)[:, None]
    j_idx = np.arange(P)[None, :]
    tri_mask = np.zeros((P, 2 * OWN), np.float32)
    for k in range(4):
        tri_mask[:, k * P:(k + 1) * P] = p_idx > j_idx
        tri_mask[:, OWN + k * P:OWN + (k + 1) * P] = p_idx <= j_idx
    tri_mask = tri_mask.astype(NPBF16)

    # validity column: ctx key c = 128*ki + p; first chunk's halo (c < 512)
    # is invalid, everything else valid
    c_ki = (np.arange(NKT)[None, :] * P + np.arange(P)[:, None])  # [P, NKT]
    v_int = np.ones((P, NKT, KVH, 1), np.float32).astype(NPBF16)
    v_first = np.broadcast_to(
        (c_ki >= OWN)[:, :, None, None], (P, NKT, KVH, 1)).astype(NPBF16)
    v_first = np.ascontiguousarray(v_first)

    shared = dict(wqT=wqT, wkvT=wkvT, woT=woT, w1T=w1T, w3T=w3T, w2T=w2T,
                  qw=qwb, kw=kwb, masks=tri_mask)
    in_maps = []
    for b in range(B):
        for j in range(T // OWN):
            xc = np.zeros((CTX, D), f32)
            if j == 0:
                xc[OWN:] = x[b, 0:OWN]
                v = v_first
            else:
                xc[:] = x[b, (j - 1) * OWN:(j + 1) * OWN]
                v = v_int
            in_maps.append(dict(shared, xtok=xc, valid=v))
    return in_maps


LAST_RESULTS = None


def _ensure_ntff_hook():
    """Install the axon NTFF profile hook if the image's antenv lacks it.

    Recreates what trn_agent_boot would register: a ctypes context manager
    around axon_{start,stop}_nrt_profile in libaxon_pjrt.so. Best-effort —
    any failure leaves tracing disabled, execution unaffected.
    """
    import types
    try:
        from antenv.axon_hooks import get_axon_ntff_profile_hook  # noqa: F401
        return  # real module present
    except ImportError:
        pass
    try:
        import antenv
        boot_dir = "/root/.axon_site/trn_agent_boot"
        if boot_dir not in sys.path:
            sys.path.insert(0, boot_dir)
        import trn_boot
        hook = trn_boot._ntff_profile_via_ctypes("/opt/axon/libaxon_pjrt.so")
        mod = types.ModuleType("antenv.axon_hooks")
        mod._hook = hook
        mod.get_axon_ntff_profile_hook = lambda: mod._hook
        mod.set_axon_ntff_profile_hook = lambda h: setattr(mod, "_hook", h)
        sys.modules["antenv.axon_hooks"] = mod
        antenv.axon_hooks = mod
        # keep profile artifacts local: no bucket upload from this container
        import concourse.bass_utils as _bu
        _bu.upload_artifacts = lambda tmpdir: tmpdir
    except Exception as e:  # pragma: no cover
        print(f"ntff hook unavailable ({e}); running without trace")


def kernel(**inputs):
    global LAST_RESULTS
    if os.environ.get("BASS_TRACE"):
        _ensure_ntff_hook()
    in_maps = prep_in_maps(inputs)
    nc = get_nc()
    res = run_bass_kernel_spmd(nc, in_maps, core_ids=list(range(NCORES)))
    LAST_RESULTS = res
    y = np.empty((B, T, D), np.float32)
    for c in range(NCORES):
        b, j = divmod(c, T // OWN)
        y[b, j * OWN:(j + 1) * OWN] = res.results[c]["y"]
    return y



# revision 43
# speedup vs baseline: 1.1034x; 1.1034x over previous
"""Trainium2 Bass kernel for a codec-transformer block (sliding-window GQA + SwiGLU).

Sharding: data-parallel over 8 token chunks (2 batches x 4 chunks of 512
tokens). The 512-token sliding window makes attention local: each core
receives its 512 "own" tokens plus the preceding 512 tokens as a KV halo,
so no collectives are needed.

Host-side prep (layout only, no model FLOPs):
  - attn_norm_w folded into wq/wk/wv columns, ffn_norm_w into w1/w3 columns
    (rmsnorm weight commutes into the following matmul exactly)
  - attn_scale folded into wo rows, ffn_scale into w2 rows (layer-scale
    commutes out of the preceding matmul exactly)
  - weights transposed to [in, out] and cast to bf16 (all matmuls run in
    bf16 with fp32 PSUM accumulation; the residual path stays fp32, and both
    branch outputs are damped by the 0.01 layer-scales so bf16 branch error
    is ~1e-4 relative on the final output)
  - per-core sliding-window masks (multiplicative, on exp(scores))

On-chip dataflow is token-major; PE transpose (via identity matmul) produces
the feature-major operands needed for stationary/moving tensors. Softmax is
computed without max-subtraction (rmsnorm'd q,k bound |score| <= 8) and
unnormalized; a ones-column appended to V yields sum(exp) in the same PSUM
tile, and the reciprocal is applied per-partition at eviction. Engine streams
execute in emission order, so norm chains are software-pipelined two tiles
ahead of their consumers to keep the PE dense (HAM stays warm).
"""

import os
import sys

sys.path.insert(0, "/opt/trn_rl_repo")
os.environ.setdefault("MYCRO_LOCAL_CACHE", "1")

from contextlib import ExitStack

import numpy as np
import ml_dtypes

import concourse.bass as bass
import concourse.bacc as bacc
import concourse.tile as tile
from concourse import mybir
from concourse.masks import make_identity
from concourse.bass_utils import run_bass_kernel_spmd

BF16 = mybir.dt.bfloat16
F32 = mybir.dt.float32
F8 = mybir.dt.float8e4
AF = mybir.ActivationFunctionType
ALU = mybir.AluOpType
DR = mybir.MatmulPerfMode.DoubleRow
NPBF16 = ml_dtypes.bfloat16
NPF8 = ml_dtypes.float8_e4m3

# fp8 scaling: weights are upscaled into e4m3's normal range (min normal
# 2^-6) and the product rescaled at eviction.  qk-norm makes q/k scale
# invariant; V/WO/FFN evictions fold the downscale into an existing fused op.
WUP = 32.0           # wq/wk/wv/w1/w3 upscale (values ~N(0,1) in fp8)
WOUP = 4096.0        # wo*attn_scale / w2*ffn_scale upscale
RES_SC = 1.0 / WOUP  # residual-add eviction scale
FT_SC = 1.0 / (WUP * WUP)  # silu(g)*u rescale (psg,psu both carry 32x)

P = 128
B, T, D = 2, 2048, 1024
HID = 4096
H, KVH, HD = 16, 4, 64
KD = D // P            # 8 contraction tiles over model dim
KH = HID // P          # 32 contraction tiles over hidden dim
OWN = 512              # tokens owned per core
CTX = 1024             # own + 512-token halo
NQT = OWN // P         # 4
NKT = CTX // P         # 8
NCORES = 8
KC = KVH * HD          # 256
EPS = 1e-5
QKEPS = 1e-6
SM_SCALE = 1.0 / 8.0   # 1/sqrt(HD)


def _qclip(ki):
    """Valid own-query range for ctx key tile ki under the sliding window."""
    return max(0, P * (ki - 4)), min(OWN, P * (ki + 1))


# Packed per-head scores layout: (ki, qlo, qhi, col_offset) blocks.  Bank 0
# holds the four diagonal (ki==qt) blocks, bank 1 the four window-edge
# (ki==qt+4) blocks, banks 2-4 the interior blocks.  Every block sits inside
# one 512-float PSUM bank.
SPACK = [
    (0, 0, 128, 0), (1, 128, 256, 128), (2, 256, 384, 256), (3, 384, 512, 384),
    (4, 0, 128, 512), (5, 128, 256, 640), (6, 256, 384, 768),
    (7, 384, 512, 896),
    (3, 0, 384, 1024), (1, 0, 128, 1408),
    (4, 128, 512, 1536), (6, 384, 512, 1920),
    (2, 0, 256, 2048), (5, 256, 512, 2304),
]
SCOL = 2560
# (ki, qt) -> packed column of that 128-wide block
SLOOKUP = {}
for _ki, _qlo, _qhi, _off in SPACK:
    for _qt in range(_qlo // P, _qhi // P):
        SLOOKUP[(_ki, _qt)] = _off + _qt * P - _qlo
assert len(SLOOKUP) == 20


def _build_tile_kernel(ctx: ExitStack, tc: tile.TileContext, io: dict):
    nc = tc.nc
    xtok, y = io["xtok"], io["y"]

    const = ctx.enter_context(tc.tile_pool(name="const", bufs=1))
    identity = const.tile([P, P], BF16)
    make_identity(nc, identity)
    qw_sb = const.tile([P, HD], F32)
    nc.sync.dma_start(qw_sb, io["qw"])
    kw_sb = const.tile([P, HD], F32)
    nc.sync.dma_start(kw_sb, io["kw"])
    eps_sb = const.tile([P, 1], F32)
    nc.vector.memset(eps_sb, EPS)
    qkeps_sb = const.tile([P, 1], F32)
    nc.vector.memset(qkeps_sb, QKEPS)
    ones_row = const.tile([1, HD], BF16)
    nc.vector.memset(ones_row, 1.0)
    zero_row = const.tile([1, OWN], BF16)
    nc.vector.memset(zero_row, 0.0)

    sstat = ctx.enter_context(tc.tile_pool(name="sstat", bufs=8))

    # persistent activations; pools are ordered so releases stay LIFO:
    # open order pers < hnT < attn_pers < xall < maskp < attnT < stage pools
    pers = ctx.enter_context(tc.tile_pool(name="pers", bufs=1))
    h_sb = pers.tile([P, NQT, D], F32)       # residual h = x + r, fp32
    hnT_pool = ctx.enter_context(tc.tile_pool(name="hnT_pool", bufs=1))
    hnT = hnT_pool.tile([P, KD, OWN], F8)
    ap_stack = ExitStack()
    attn_pers = ap_stack.enter_context(tc.tile_pool(name="attn_pers", bufs=1))
    # qhat^T: q heads are laid out (via the host-side wq column permutation)
    # so head h lives in feature tile tau=(h%4)+4*(h//8) at partition base
    # pi=((h//4)%2)*64 -- exactly where its kv head lands in kT2's natural
    # pair-transpose layout, so scores operands always share a base partition.
    qkT = attn_pers.tile([P, KD, OWN], BF16)
    # khat^T: pair-transposed [2 kv heads x 64, ctx] per feature tile
    kT2 = attn_pers.tile([P, 2, CTX], BF16)
    # v tokens + per-key VALIDITY column: halo keys of the first chunk carry
    # validity 0, so they drop out of both the PV numerator (v rows are zero
    # there anyway) and the softmax denominator -- no per-chunk masks needed.
    v65 = attn_pers.tile([P, NKT, KVH, HD + 1], BF16)
    nc.sync.dma_start(v65[:, :, :, HD:HD + 1], io["valid"])

    # Prefetch the whole x slice up front: 8 parallel DMAs overlap with the
    # first norm chains; the own-token tiles also serve the stage-D residual.
    xall_stack = ExitStack()
    xall_pool = xall_stack.enter_context(tc.tile_pool(name="xall", bufs=1))
    xall = xall_pool.tile([P, NKT, D], F32)
    for i in range(NKT):
        nc.sync.dma_start(xall[:, i, :], xtok[i * P:(i + 1) * P, :])

    # ---- Stages A+B: rmsnorm + xnT transpose + QKV, per ctx tile ----
    stage_a = ExitStack()
    with stage_a:
        wqkv_pool = stage_a.enter_context(tc.tile_pool(name="wqkv", bufs=1))
        wkv_sb = wqkv_pool.tile([P, KD, 2 * KC], F8)
        nc.sync.dma_start(wkv_sb, io["wkvT"].rearrange("(kd p) n -> p kd n", p=P))
        wq_sb = wqkv_pool.tile([P, KD, D], F8)
        nc.sync.dma_start(wq_sb, io["wqT"].rearrange("(kd p) n -> p kd n", p=P))

        pa = stage_a.enter_context(tc.tile_pool(name="pa", bufs=3))
        tp_ps = stage_a.enter_context(
            tc.tile_pool(name="tp_ps", bufs=4, space="PSUM"))
        xnT_pool = stage_a.enter_context(tc.tile_pool(name="xnT_pool", bufs=1))
        xnT = xnT_pool.tile([P, KD, CTX], F8)
        pb_ps = stage_a.enter_context(
            tc.tile_pool(name="pb_ps", bufs=3, space="PSUM"))
        pb = stage_a.enter_context(tc.tile_pool(name="pb", bufs=2))

        def norm_tile(i):
            """rmsnorm of x tile i -> bf16 xn tile (ACT/DVE only)."""
            xt = xall[:, i, :]
            sq = pa.tile([P, D], F32, tag="sq")
            ssq = sstat.tile([P, 1], F32, tag="ssq")
            nc.scalar.activation(sq, xt, AF.Square, accum_out=ssq)
            std = sstat.tile([P, 1], F32, tag="std")
            nc.scalar.activation(std, ssq, AF.Sqrt, bias=eps_sb, scale=1.0 / D)
            rstd = sstat.tile([P, 1], F32, tag="rstd")
            nc.vector.reciprocal(rstd, std)
            xn = pa.tile([P, D], BF16, tag="xn")
            nc.vector.tensor_scalar_mul(xn, xt, rstd)
            return xn

        def emit_k_tp(kt, khat):
            # pair transpose: kv heads 2kf (base 0) and 2kf+1 (base 64)
            for kf in range(2):
                pt = tp_ps.tile([P, P], BF16, tag="tp")
                nc.tensor.transpose(pt, khat[:, kf * P:(kf + 1) * P], identity)
                nc.vector.tensor_copy(kT2[:, kf, kt * P:(kt + 1) * P], pt)

        def emit_q_tp(qt, qhats):
            for half in range(2):
                for j in range(4):
                    pt = tp_ps.tile([P, P], BF16, tag="tp")
                    nc.tensor.transpose(
                        pt, qhats[half][:, j * P:(j + 1) * P], identity)
                    nc.vector.tensor_copy(
                        qkT[:, half * 4 + j, qt * P:(qt + 1) * P], pt)

        # Norm chains run two tiles ahead of the PE; k/q-hat transposes run
        # one tile BEHIND their matmuls so the qk-norm ACT/DVE chains finish
        # in the shadow of the next tile's matmuls.
        xns = {0: norm_tile(0), 1: norm_tile(1)}
        pending_k = None
        pending_q = None
        for i in range(NKT):
            xn = xns.pop(i)
            for kd in range(KD):
                pt = tp_ps.tile([P, P], BF16, tag="tp")
                nc.tensor.transpose(pt, xn[:, kd * P:(kd + 1) * P], identity)
                nc.vector.tensor_copy(xnT[:, kd, i * P:(i + 1) * P], pt)
            if i + 2 < NKT:
                xns[i + 2] = norm_tile(i + 2)

            # K / V projection for ctx tile i (fp8 DoubleRow: 2 kd per MM)
            kt = i
            ps = pb_ps.tile([P, 512], F32, tag="ps")
            for kp in range(KD // 2):
                nc.tensor.matmul(
                    ps, lhsT=xnT[:, 2 * kp:2 * kp + 2, kt * P:(kt + 1) * P],
                    rhs=wkv_sb[:, 2 * kp:2 * kp + 2, :],
                    start=(kp == 0), stop=(kp == KD // 2 - 1), perf_mode=DR)
            if pending_k is not None:
                emit_k_tp(*pending_k)
            sqk = pb.tile([P, KC], F32, tag="sqk")
            nc.scalar.activation(sqk, ps[:, 0:KC], AF.Square)
            msk = pb.tile([P, KVH], F32, tag="msk")
            nc.vector.reduce_sum(
                msk, sqk.rearrange("p (h e) -> p h e", e=HD),
                axis=mybir.AxisListType.X)
            sck = sstat.tile([P, KVH], F32, tag="sck")
            nc.scalar.activation(sck, msk, AF.Sqrt, bias=qkeps_sb, scale=1.0 / HD)
            rck = sstat.tile([P, KVH], F32, tag="rck")
            nc.vector.reciprocal(rck, sck)
            tmk = pb.tile([P, KC], F32, tag="tmk")
            nc.vector.tensor_mul(
                tmk.rearrange("p (h e) -> p h e", e=HD),
                ps[:, 0:KC].rearrange("p (h e) -> p h e", e=HD),
                kw_sb[:, None, :].broadcast_to([P, KVH, HD]))
            khat = pb.tile([P, KC], BF16, tag="khat")
            nc.vector.tensor_mul(
                khat.rearrange("p (h e) -> p h e", e=HD),
                tmk.rearrange("p (h e) -> p h e", e=HD),
                rck[:, :, None].broadcast_to([P, KVH, HD]))
            pending_k = (kt, khat)
            nc.scalar.activation(
                v65[:, kt, :, 0:HD],
                ps[:, KC:2 * KC].rearrange("p (h e) -> p h e", e=HD),
                AF.Copy, scale=1.0 / WUP)

            # Q projection + qk-norm for own tile qt = i - 4
            if i < NQT:
                continue
            qt = i - NQT
            col = OWN + qt * P
            msq = pb.tile([P, H], F32, tag="msq")
            pss = []
            for half in range(2):
                ps = pb_ps.tile([P, 512], F32, tag="ps")
                pss.append(ps)
                for kp in range(KD // 2):
                    nc.tensor.matmul(
                        ps, lhsT=xnT[:, 2 * kp:2 * kp + 2, col:col + P],
                        rhs=wq_sb[:, 2 * kp:2 * kp + 2,
                                  half * 512:(half + 1) * 512],
                        start=(kp == 0), stop=(kp == KD // 2 - 1),
                        perf_mode=DR)
                sqq = pb.tile([P, 512], F32, tag="sqq")
                nc.scalar.activation(sqq, ps, AF.Square)
                nc.vector.reduce_sum(
                    msq[:, half * 8:(half + 1) * 8],
                    sqq.rearrange("p (h e) -> p h e", e=HD),
                    axis=mybir.AxisListType.X)
            if pending_q is not None:
                emit_q_tp(*pending_q)
            sc = sstat.tile([P, H], F32, tag="sc")
            nc.scalar.activation(sc, msq, AF.Sqrt, bias=qkeps_sb, scale=1.0 / HD)
            rc = sstat.tile([P, H], F32, tag="rc")
            nc.vector.reciprocal(rc, sc)
            qhats = []
            for half in range(2):
                ps = pss[half]
                tmq = pb.tile([P, 512], F32, tag="tmq")
                nc.vector.tensor_mul(
                    tmq.rearrange("p (h e) -> p h e", e=HD),
                    ps.rearrange("p (h e) -> p h e", e=HD),
                    qw_sb[:, None, :].broadcast_to([P, 8, HD]))
                qhat = pb.tile([P, 512], BF16, tag="qhat")
                nc.vector.tensor_mul(
                    qhat.rearrange("p (h e) -> p h e", e=HD),
                    tmq.rearrange("p (h e) -> p h e", e=HD),
                    rc[:, half * 8:(half + 1) * 8, None].broadcast_to([P, 8, HD]))
                qhats.append(qhat)
            pending_q = (qt, qhats)
        emit_k_tp(*pending_k)
        emit_q_tp(*pending_q)

    # ---- Stage C: attention (packed scores PSUM + PV-swap) ----
    # Scores for one head are packed into a single 5-bank PSUM tile so exp
    # runs as 2 big ACT ops (the 352-cycle ACT overhead dominated the old
    # per-ki exp).  Layout regions: bank0 = the 4 diagonal (ki==qt) blocks,
    # bank1 = the 4 window-edge (ki==qt+4) blocks, banks 2-4 = interior
    # blocks (always fully valid for interior chunks).  One full-width mask
    # multiply per head handles both boundary triangles and the chunk-0 halo.
    # PV is swapped vs the baseline: v65 is the stationary operand and eS the
    # moving one, so the output lands directly feature-major as attnT (no PE
    # transposes) and the per-MM cost drops from ~110ns (LDW-bound, N=65) to
    # ~60ns (N=128).  The ones column of v65 lands in PSUM partition 64; its
    # reciprocal (DVE, also the PSUM->SBUF move) is partition-broadcast by
    # GpSimd and applied in a single [64,512] eviction multiply per head.
    mask_stack = ExitStack()
    maskp = mask_stack.enter_context(tc.tile_pool(name="maskp", bufs=1))
    masks_sb = maskp.tile([P, 2 * OWN], BF16)
    nc.sync.dma_start(masks_sb, io["masks"])
    attnT_stack = ExitStack()
    attnT_pool = attnT_stack.enter_context(tc.tile_pool(name="attnT_pool",
                                                        bufs=1))
    attnT = attnT_pool.tile([P, KD, OWN], F8)

    stage_c = ExitStack()
    with stage_c:
        pc_ex = stage_c.enter_context(tc.tile_pool(name="pc_ex", bufs=2))
        pc_es = stage_c.enter_context(tc.tile_pool(name="pc_es", bufs=2))
        pc_rec = stage_c.enter_context(tc.tile_pool(name="pc_rec", bufs=2))
        ps_s = stage_c.enter_context(
            tc.tile_pool(name="ps_s", bufs=1, space="PSUM"))
        ps_o = stage_c.enter_context(
            tc.tile_pool(name="ps_o", bufs=2, space="PSUM"))
        ps_b = stage_c.enter_context(
            tc.tile_pool(name="ps_b", bufs=1, space="PSUM"))
        S = ps_s.tile([P, SCOL], F32)  # 5 PSUM banks, reused across heads

        def emit_warm():
            # Full-width dummy matmul (K=128, N=512).  The attention MM mix
            # (64-deep scores, N=128 PV) never trips HAM's activity monitor,
            # so the whole phase runs at 1.2 GHz; a few dense matmuls per
            # head keep the clock gate at 8/8.
            wps = ps_b.tile([P, OWN], F32, tag="sb")
            nc.tensor.matmul(wps, lhsT=identity, rhs=masks_sb[:, 0:OWN],
                             start=True, stop=True)

        def emit_scores(h, S):
            tau = (h % 4) + 4 * (h // 8)
            kf = (h // 4) // 2
            pi = ((h // 4) % 2) * HD
            for n, (ki, qlo, qhi, off) in enumerate(SPACK):
                w = qhi - qlo
                if n == 8:
                    emit_warm()
                nc.tensor.matmul(
                    S[:, off:off + w],
                    lhsT=kT2[pi:pi + HD, kf, ki * P:(ki + 1) * P],
                    rhs=qkT[pi:pi + HD, tau, qlo:qhi],
                    start=True, stop=True)
            emit_warm()

        def emit_exp_mask(h, S):
            # diag+edge regions [0:1024) get the (constant) triangle mask;
            # interior regions exp straight into eS
            ex = pc_ex.tile([P, 2 * OWN], BF16, tag="ex")
            eS = pc_es.tile([P, SCOL], BF16, tag="eS")
            nc.scalar.activation(ex, S[:, 0:2 * OWN], AF.Exp, scale=SM_SCALE)
            nc.scalar.activation(eS[:, 2 * OWN:SCOL], S[:, 2 * OWN:SCOL],
                                 AF.Exp, scale=SM_SCALE)
            nc.vector.tensor_mul(eS[:, 0:2 * OWN], ex, masks_sb)
            return eS

        def emit_pv(h, eS):
            kvh = h // 4
            tau = (h % 4) + 4 * (h // 8)
            slot = 2 * tau + ((h // 4) % 2)
            pi = (slot % 2) * HD
            kd = slot // 2
            # ki-major with merged interior blocks.  start=True resets the
            # whole 2KB PSUM bank (not just the written window), so a K=1
            # zeroing matmul opens the bank once and every PV matmul
            # accumulates; only the last carries stop.
            po = ps_o.tile([P, OWN], F32, tag="po")
            nc.tensor.matmul(po[0:HD + 1, :], lhsT=zero_row[0:1, 0:HD + 1],
                             rhs=zero_row[0:1, :], start=True, stop=False,
                             skip_group_check=True)
            for ki in range(NKT):
                if ki == 4:
                    emit_warm()
                if ki <= 3:  # diagonal block (window qt=ki)
                    nc.tensor.matmul(
                        po[0:HD + 1, ki * P:(ki + 1) * P],
                        lhsT=v65[:, ki, kvh, :],
                        rhs=eS[:, SLOOKUP[(ki, ki)]:SLOOKUP[(ki, ki)] + P],
                        start=False, stop=False, skip_group_check=True)
                if ki <= 3:        # interior = qt 0..ki-1 (diag qt=ki is last)
                    ilo, ihi = 0, ki * P
                else:              # interior = qt ki-3..3 (edge qt=ki-4 first)
                    ilo, ihi = (ki - 3) * P, OWN
                if ihi > ilo:
                    col = SLOOKUP[(ki, ilo // P)]
                    nc.tensor.matmul(
                        po[0:HD + 1, ilo:ihi],
                        lhsT=v65[:, ki, kvh, :],
                        rhs=eS[:, col:col + ihi - ilo],
                        start=False, stop=False, skip_group_check=True)
                if ki >= 4:  # edge block (window qt=ki-4)
                    qt = ki - 4
                    nc.tensor.matmul(
                        po[0:HD + 1, qt * P:(qt + 1) * P],
                        lhsT=v65[:, ki, kvh, :],
                        rhs=eS[:, SLOOKUP[(ki, qt)]:SLOOKUP[(ki, qt)] + P],
                        start=False, stop=(ki == NKT - 1),
                        skip_group_check=True)
            # sums row -> SBUF (bf16), PE K=1 broadcast to 64 partitions,
            # reciprocal back to SBUF, one multiply into attnT
            srow = pc_rec.tile([1, OWN], BF16, tag="srow")
            nc.vector.tensor_copy(srow[0:1, :], po[HD:HD + 1, :])
            sb = ps_b.tile([P, OWN], F32, tag="sb")
            nc.tensor.matmul(sb[0:HD, :], lhsT=ones_row[0:1, :],
                             rhs=srow[0:1, :], start=True, stop=True)
            rec_b = pc_rec.tile([HD, OWN], F32, tag="rec_b")
            nc.vector.reciprocal_approx_fast(out=rec_b, in_=sb[0:HD, :])
            nc.vector.tensor_mul(attnT[pi:pi + HD, kd, :], po[0:HD, :], rec_b)

        pending = None
        for h in range(H):
            emit_scores(h, S)
            eS = emit_exp_mask(h, S)
            if pending is not None:
                emit_pv(*pending)
            pending = (h, eS)
        emit_pv(*pending)

    # ---- Stages D+E: output projection + residual + ffn norm, per qt ----
    stage_de = ExitStack()
    with stage_de:
        wo_pool = stage_de.enter_context(tc.tile_pool(name="wo_pool", bufs=1))
        wo_sb = wo_pool.tile([P, KD, D], F8)
        nc.sync.dma_start(wo_sb, io["woT"].rearrange("(kd p) n -> p kd n", p=P))
        ps_r = stage_de.enter_context(
            tc.tile_pool(name="ps_r", bufs=2, space="PSUM"))
        tp_ps3 = stage_de.enter_context(
            tc.tile_pool(name="tp_ps3", bufs=2, space="PSUM"))
        pe = stage_de.enter_context(tc.tile_pool(name="pe", bufs=2))

        for qt in range(NQT):
            xr = xall[:, NQT + qt, :]
            for half in range(2):
                ps = ps_r.tile([P, 512], F32, tag="psr")
                for kp in range(KD // 2):
                    nc.tensor.matmul(
                        ps, lhsT=attnT[:, 2 * kp:2 * kp + 2, qt * P:(qt + 1) * P],
                        rhs=wo_sb[:, 2 * kp:2 * kp + 2,
                                  half * 512:(half + 1) * 512],
                        start=(kp == 0), stop=(kp == KD // 2 - 1),
                        perf_mode=DR)
                # h = ps * (attn_scale/WOUP already in wo) ... = ps/WOUP + x
                nc.vector.scalar_tensor_tensor(
                    h_sb[:, qt, half * 512:(half + 1) * 512], ps, RES_SC,
                    xr[:, half * 512:(half + 1) * 512], ALU.mult, ALU.add)
            # ffn rmsnorm for this qt (overlaps next qt's wo matmuls)
            sqh = pe.tile([P, D], F32, tag="sqh")
            ssqh = sstat.tile([P, 1], F32, tag="ssq")
            nc.scalar.activation(sqh, h_sb[:, qt, :], AF.Square, accum_out=ssqh)
            stdh = sstat.tile([P, 1], F32, tag="std")
            nc.scalar.activation(stdh, ssqh, AF.Sqrt, bias=eps_sb, scale=1.0 / D)
            rstdh = sstat.tile([P, 1], F32, tag="rstd")
            nc.vector.reciprocal(rstdh, stdh)
            hn = pe.tile([P, D], BF16, tag="hn")
            nc.vector.tensor_scalar_mul(hn, h_sb[:, qt, :], rstdh)
            for kd in range(KD):
                pt = tp_ps3.tile([P, P], BF16, tag="tp3")
                nc.tensor.transpose(pt, hn[:, kd * P:(kd + 1) * P], identity)
                nc.vector.tensor_copy(hnT[:, kd, qt * P:(qt + 1) * P], pt)

    attnT_stack.close()
    mask_stack.close()
    xall_stack.close()
    ap_stack.close()

    # ---- Stage F: SwiGLU FFN ----
    stage_f = ExitStack()
    with stage_f:
        fT_pool = stage_f.enter_context(tc.tile_pool(name="fT_pool", bufs=1))
        fT = fT_pool.tile([P, KH, OWN], F8)     # (silu(g) * u)^T feature-major
        w13 = stage_f.enter_context(tc.tile_pool(name="w13", bufs=4))
        ps_f = stage_f.enter_context(
            tc.tile_pool(name="ps_f", bufs=2, space="PSUM"))
        pf = stage_f.enter_context(tc.tile_pool(name="pf", bufs=2))

        for mi in range(KH):
            w1t = w13.tile([P, KD, P], F8, tag="w1t")
            nc.sync.dma_start(
                w1t, io["w1T"][:, mi * P:(mi + 1) * P]
                .rearrange("(kd p) m -> p kd m", p=P))
            w3t = w13.tile([P, KD, P], F8, tag="w3t")
            nc.sync.dma_start(
                w3t, io["w3T"][:, mi * P:(mi + 1) * P]
                .rearrange("(kd p) m -> p kd m", p=P))
            psg = ps_f.tile([P, 512], F32, tag="pg")
            for kp in range(KD // 2):
                nc.tensor.matmul(psg, lhsT=w1t[:, 2 * kp:2 * kp + 2, :],
                                 rhs=hnT[:, 2 * kp:2 * kp + 2, :],
                                 start=(kp == 0), stop=(kp == KD // 2 - 1),
                                 perf_mode=DR)
            psu = ps_f.tile([P, 512], F32, tag="pu")
            for kp in range(KD // 2):
                nc.tensor.matmul(psu, lhsT=w3t[:, 2 * kp:2 * kp + 2, :],
                                 rhs=hnT[:, 2 * kp:2 * kp + 2, :],
                                 start=(kp == 0), stop=(kp == KD // 2 - 1),
                                 perf_mode=DR)
            # silu(g)*u via sigmoid; psg/psu carry the WUP upscale: the STT
            # folds the full 1/WUP^2 rescale into gm (tensor_tensor_reduce
            # with fp8 output crashes the device, so rescale upstream)
            sg = pf.tile([P, 512], F32, tag="sg")
            nc.scalar.activation(sg, psg, AF.Sigmoid, scale=1.0 / WUP)
            gm = pf.tile([P, 512], BF16, tag="gm")
            nc.vector.scalar_tensor_tensor(gm, psg, FT_SC, sg, ALU.mult,
                                           ALU.mult)
            nc.vector.tensor_mul(fT[:, mi, :], gm, psu)

        w2_pool = stage_f.enter_context(tc.tile_pool(name="w2_pool", bufs=1))
        w2_sb = w2_pool.tile([P, KH, D], F8)
        # 8 chunked DMAs spread across queues (a single 8MB transfer
        # serializes on one queue and stalls the w2 matmuls ~15us)
        for c in range(8):
            nc.sync.dma_start(
                w2_sb[:, c * 4:(c + 1) * 4, :],
                io["w2T"][c * 4 * P:(c + 1) * 4 * P, :]
                .rearrange("(kh p) n -> p kh n", p=P))
        ps_y = stage_f.enter_context(
            tc.tile_pool(name="ps_y", bufs=2, space="PSUM"))
        py = stage_f.enter_context(tc.tile_pool(name="py", bufs=2))

        for qt in range(NQT):
            yt = py.tile([P, D], F32, tag="yt")
            for half in range(2):
                ps = ps_y.tile([P, 512], F32, tag="psy")
                for kp in range(KH // 2):
                    nc.tensor.matmul(
                        ps, lhsT=fT[:, 2 * kp:2 * kp + 2, qt * P:(qt + 1) * P],
                        rhs=w2_sb[:, 2 * kp:2 * kp + 2,
                                  half * 512:(half + 1) * 512],
                        start=(kp == 0), stop=(kp == KH // 2 - 1),
                        perf_mode=DR)
                nc.vector.scalar_tensor_tensor(
                    yt[:, half * 512:(half + 1) * 512], ps, RES_SC,
                    h_sb[:, qt, half * 512:(half + 1) * 512],
                    ALU.mult, ALU.add)
            nc.sync.dma_start(y[qt * P:(qt + 1) * P, :], yt)


def build_nc():
    nc = bacc.Bacc("TRN2", target_bir_lowering=False, debug=False,
                   num_devices=NCORES)
    io = {
        "xtok": nc.dram_tensor("xtok", [CTX, D], F32, kind="ExternalInput").ap(),
        "wqT": nc.dram_tensor("wqT", [D, D], F8, kind="ExternalInput").ap(),
        "wkvT": nc.dram_tensor("wkvT", [D, 2 * KVH * HD], F8,
                               kind="ExternalInput").ap(),
        "woT": nc.dram_tensor("woT", [D, D], F8, kind="ExternalInput").ap(),
        "w1T": nc.dram_tensor("w1T", [D, HID], F8, kind="ExternalInput").ap(),
        "w3T": nc.dram_tensor("w3T", [D, HID], F8, kind="ExternalInput").ap(),
        "w2T": nc.dram_tensor("w2T", [HID, D], F8, kind="ExternalInput").ap(),
        "qw": nc.dram_tensor("qw", [P, HD], F32, kind="ExternalInput").ap(),
        "kw": nc.dram_tensor("kw", [P, HD], F32, kind="ExternalInput").ap(),
        "masks": nc.dram_tensor("masks", [P, 2 * OWN], BF16,
                                kind="ExternalInput").ap(),
        "valid": nc.dram_tensor("valid", [P, NKT, KVH, 1], BF16,
                                kind="ExternalInput").ap(),
        "y": nc.dram_tensor("y", [OWN, D], F32, kind="ExternalOutput").ap(),
    }
    with tile.TileContext(nc) as tc:
        with ExitStack() as ctx:
            _build_tile_kernel(ctx, tc, io)
    nc.compile()
    return nc


_CACHE = {}


def get_nc():
    if "nc" not in _CACHE:
        _CACHE["nc"] = build_nc()
    return _CACHE["nc"]


def prep_in_maps(inputs):
    """Fold scales into weights, transpose/cast, and slice per-core inputs."""
    f32 = np.float32
    x = np.asarray(inputs["x"], f32)
    wq = np.asarray(inputs["wq"], f32)
    wk = np.asarray(inputs["wk"], f32)
    wv = np.asarray(inputs["wv"], f32)
    wo = np.asarray(inputs["wo"], f32)
    w1 = np.asarray(inputs["w1"], f32)
    w2 = np.asarray(inputs["w2"], f32)
    w3 = np.asarray(inputs["w3"], f32)
    qw = np.asarray(inputs["q_norm_w"], f32)
    kw = np.asarray(inputs["k_norm_w"], f32)
    anw = np.asarray(inputs["attn_norm_w"], f32)
    fnw = np.asarray(inputs["ffn_norm_w"], f32)
    asc = np.asarray(inputs["attn_scale"], f32)
    fsc = np.asarray(inputs["ffn_scale"], f32)

    # q-head permutation: slot j of the on-chip q/attn feature layout holds
    # head HEAD_PERM[j], so each q head's partition half matches its kv
    # head's natural pair-transpose position (see kernel layout comment)
    HEAD_PERM = [0, 4, 1, 5, 2, 6, 3, 7, 8, 12, 9, 13, 10, 14, 11, 15]
    wq_p = (wq * anw[None, :] * WUP).reshape(H, HD, D)[HEAD_PERM] \
        .reshape(H * HD, D)
    wqT = np.ascontiguousarray(wq_p.T).astype(NPF8)
    wkvT = np.ascontiguousarray(
        np.concatenate([wk * anw[None, :], wv * anw[None, :]], axis=0).T * WUP
    ).astype(NPF8)
    wo_p = (wo * asc[:, None] * WOUP).T.reshape(H, HD, D)[HEAD_PERM] \
        .reshape(H * HD, D)
    woT = np.ascontiguousarray(wo_p).astype(NPF8)
    w1T = np.ascontiguousarray((w1 * fnw[None, :] * WUP).T).astype(NPF8)
    w3T = np.ascontiguousarray((w3 * fnw[None, :] * WUP).T).astype(NPF8)
    w2T = np.ascontiguousarray((w2 * fsc[:, None] * WOUP).T).astype(NPF8)
    qwb = np.ascontiguousarray(np.broadcast_to(qw[None, :], (P, HD))).astype(f32)
    kwb = np.ascontiguousarray(np.broadcast_to(kw[None, :], (P, HD))).astype(f32)

    # Boundary mask for the packed diag+edge regions (identical across cores
    # and chunks): diag blocks (ki==qt) are the strict-causal triangle p > j,
    # edge blocks (ki==qt+4) are the window triangle p <= j.  Chunk-0 halo
    # invalidity is handled by the v65 validity column, not the mask.
    p_idx = np.arange(P)[:, None]
    j_idx = np.arange(P)[None, :]
    tri_mask = np.zeros((P, 2 * OWN), np.float32)
    for k in range(4):
        tri_mask[:, k * P:(k + 1) * P] = p_idx > j_idx
        tri_mask[:, OWN + k * P:OWN + (k + 1) * P] = p_idx <= j_idx
    tri_mask = tri_mask.astype(NPBF16)

    # validity column: ctx key c = 128*ki + p; first chunk's halo (c < 512)
    # is invalid, everything else valid
    c_ki = (np.arange(NKT)[None, :] * P + np.arange(P)[:, None])  # [P, NKT]
    v_int = np.ones((P, NKT, KVH, 1), np.float32).astype(NPBF16)
    v_first = np.broadcast_to(
        (c_ki >= OWN)[:, :, None, None], (P, NKT, KVH, 1)).astype(NPBF16)
    v_first = np.ascontiguousarray(v_first)

    shared = dict(wqT=wqT, wkvT=wkvT, woT=woT, w1T=w1T, w3T=w3T, w2T=w2T,
                  qw=qwb, kw=kwb, masks=tri_mask)
    in_maps = []
    for b in range(B):
        for j in range(T // OWN):
            xc = np.zeros((CTX, D), f32)
            if j == 0:
                xc[OWN:] = x[b, 0:OWN]
                v = v_first
            else:
                xc[:] = x[b, (j - 1) * OWN:(j + 1) * OWN]
                v = v_int
            in_maps.append(dict(shared, xtok=xc, valid=v))
    return in_maps


LAST_RESULTS = None


def _ensure_ntff_hook():
    """Install the axon NTFF profile hook if the image's antenv lacks it.

    Recreates what trn_agent_boot would register: a ctypes context manager
    around axon_{start,stop}_nrt_profile in libaxon_pjrt.so. Best-effort —
    any failure leaves tracing disabled, execution unaffected.
    """
    import types
    try:
        from antenv.axon_hooks import get_axon_ntff_profile_hook  # noqa: F401
        return  # real module present
    except ImportError:
        pass
    try:
        import antenv
        boot_dir = "/root/.axon_site/trn_agent_boot"
        if boot_dir not in sys.path:
            sys.path.insert(0, boot_dir)
        import trn_boot
        hook = trn_boot._ntff_profile_via_ctypes("/opt/axon/libaxon_pjrt.so")
        mod = types.ModuleType("antenv.axon_hooks")
        mod._hook = hook
        mod.get_axon_ntff_profile_hook = lambda: mod._hook
        mod.set_axon_ntff_profile_hook = lambda h: setattr(mod, "_hook", h)
        sys.modules["antenv.axon_hooks"] = mod
        antenv.axon_hooks = mod
        # keep profile artifacts local: no bucket upload from this container
        import concourse.bass_utils as _bu
        _bu.upload_artifacts = lambda tmpdir: tmpdir
    except Exception as e:  # pragma: no cover
        print(f"ntff hook unavailable ({e}); running without trace")


def kernel(**inputs):
    global LAST_RESULTS
    if os.environ.get("BASS_TRACE"):
        _ensure_ntff_hook()
    in_maps = prep_in_maps(inputs)
    nc = get_nc()
    res = run_bass_kernel_spmd(nc, in_maps, core_ids=list(range(NCORES)))
    LAST_RESULTS = res
    y = np.empty((B, T, D), np.float32)
    for c in range(NCORES):
        b, j = divmod(c, T // OWN)
        y[b, j * OWN:(j + 1) * OWN] = res.results[c]["y"]
    return y

